# revision 1
# baseline (speedup 1.0000x reference)
"""Trainium2 raw-Bass kernel for nn_InteractionPruningLayer (sparse_attention).

Math (B=1024, F=256, D=64):
    qkv   = einsum('fd,nde->nfe', indicator, W_qkv)            # [3,F,D]
    gate  = (trans[0] @ trans[1].T > 0);  G = (qkv1 @ qkv0.T) * gate
    s[n,b,f] = feature[b,f,:] . qkv[n,f,:];  t = s0*s2;  u = s1
    out[b,i,:] = t[b,i] * sum_j u[b,j] * G[i,j] * qkv2[j,:]

Split of work:
    host   — weight prep (G, qkv2) and the per-(b,f) projections s/t/u
             (~1.7% of FLOPs, done in f32; the axon tunnel at ~50MB/s
             makes shipping the 67MB feature the bottleneck, while t/u
             are 2MB)
    device — 8 cores, batch-parallel, 128 rows each: the O(B*F^2*D)
             gated interaction contraction (~98% of FLOPs) producing
             the full [B,F,D] output:
                 K2[j,(i,d)] = G[i,j] * qkv2[j,d]          (built on-chip)
                 inner[b,(i,d)] = sum_j uT[j,b] * K2[j,(i,d)]
                 out[b,(i,d)] = t[b,i] * inner[b,(i,d)]    (stored bf16)

Raw bass blocks + explicit semaphores (Tile-emitted multi-wait sync does
not codegen under this walrus build). All bulk wire tensors are bf16;
the bf16 output is upcast to f32 on the host. A persistent jax
compilation cache avoids the ~0.7s/call re-lowering the fresh jax.jit
inside run_bass_via_pjrt would otherwise pay.
"""

import numpy as np
import ml_dtypes

B, F, D = 1024, 256, 64
NCORES = 8
BL = B // NCORES
FD = F * D                 # 16384
NCH = 16                   # main-mm chunks of 1024
_compiled = None


def _setup_jax_cache():
    import jax
    try:
        if jax.config.jax_compilation_cache_dir is None:
            jax.config.update("jax_compilation_cache_dir",
                              "/tmp/bass_jax_cache")
            jax.config.update("jax_persistent_cache_min_entry_size_bytes", -1)
            jax.config.update("jax_persistent_cache_min_compile_time_secs", 0)
    except Exception:
        pass


def _host_precompute(indicator, W_qk, W_qkv):
    """consts [128, 640] bf16: GT (2x[128,256] halves) + qkv2 (2x[128,64]).
    Also returns qkv [3,F,D] f32 for the host-side s projections."""
    indicator = np.asarray(indicator)
    W_qk = np.asarray(W_qk)
    W_qkv = np.asarray(W_qkv)
    ind = indicator.astype(np.float32)
    qkv = np.einsum('fd,nde->nfe', ind, W_qkv.astype(np.float32))
    trans = np.einsum('fd,nde->nfe', ind, W_qk.astype(np.float32))
    gate = (trans[0] @ trans[1].T) > 0
    G = np.where(gate, qkv[1] @ qkv[0].T, np.float32(0.0)).astype(np.float32)
    GT = np.ascontiguousarray(G.T)                       # [j, i]
    consts = np.zeros((128, 640), dtype=np.float32)
    consts[:, 0:256] = GT[0:128]
    consts[:, 256:512] = GT[128:256]
    consts[:, 512:576] = qkv[2][0:128]
    consts[:, 576:640] = qkv[2][128:256]
    return consts.astype(ml_dtypes.bfloat16), qkv


def _host_tu(feature, qkv):
    """t = s0*s2 (f32 [B,F]), uT packed per-core [128,(jc,b)] bf16."""
    f = np.asarray(feature, dtype=np.float32)
    s = np.einsum('bfd,nfd->nbf', f, qkv, optimize=True)
    t = (s[0] * s[2]).astype(np.float32)                 # [B, F]
    u = s[1].astype(ml_dtypes.bfloat16)                  # [B, F]
    # uT[core][j_local, jc*128 + b] = u[core*128 + b, jc*128 + j_local]
    uT = np.ascontiguousarray(
        u.reshape(NCORES, 128, 2, 128).transpose(0, 3, 2, 1)
        .reshape(NCORES, 128, 256))
    return t, uT


def _build_bass():
    import concourse.bass as bass
    from concourse import mybir

    nc = bass.Bass()
    f32, bf16 = mybir.dt.float32, mybir.dt.bfloat16

    const_d = nc.declare_dram_parameter("consts", [128, 640], bf16, isOutput=False)
    t_d = nc.declare_dram_parameter("tvec", [128, 256], f32, isOutput=False)
    u_d = nc.declare_dram_parameter("uT", [128, 256], bf16, isOutput=False)
    out_d = nc.declare_dram_parameter("out", [BL, FD], bf16, isOutput=True)

    consts = nc.alloc_sbuf_tensor("consts_sb", [128, 640], bf16).ap()
    k2 = nc.alloc_sbuf_tensor("k2", [128, 2 * FD], bf16).ap()   # [j, (jc,i,d)]
    grep = nc.alloc_sbuf_tensor("grep", [128, 2, 4096], bf16).ap()
    t_sb = nc.alloc_sbuf_tensor("t_sb", [128, 256], f32).ap()
    uT = nc.alloc_sbuf_tensor("uT_sb", [128, 2, 128], bf16).ap()
    osb = nc.alloc_sbuf_tensor("osb", [128, FD], bf16).ap()
    mp = [nc.alloc_psum_tensor(f"mp{i}", [128, 1024], f32).ap() for i in range(2)]

    gt = consts[:, 0:512].rearrange("k (c i) -> k c i", c=2)
    qkv2 = consts[:, 512:640].rearrange("k (c d) -> k c d", c=2)
    k2q = k2.rearrange("k (q x) -> k q x", q=8)
    t3 = t_sb.rearrange("b (i x) -> b i x", x=1)

    with (
        nc.Block() as block,
        nc.semaphore("sL") as sL,
        nc.semaphore("sA") as sA,
        nc.semaphore("sK") as sK,
        nc.semaphore("sM") as sM,
        nc.semaphore("sE") as sE,
        nc.semaphore("sO") as sO,
    ):
        # Semaphore state can survive across executions on these
        # long-lived axon terminals (alloc does NOT clear). Each engine
        # zeroes the sems it waits on first; gpsimd delays its first DMA
        # (~40us of NOPs) so no increment can precede the clears.
        @block.gpsimd
        def _(g):
            g.sem_clear(sE)
            g.sem_clear(sO)
            for _ in range(2):
                g.nop(cycle_cnt=30000)
            g.dma_start(out=consts[:], in_=const_d[:]).then_inc(sL, 16)
            g.dma_start(out=t_sb[:], in_=t_d[:]).then_inc(sL, 16)
            g.dma_start(out=uT[:], in_=u_d.rearrange("k (c b) -> k c b", c=2)
                        ).then_inc(sL, 16)
            for c in range(4):
                g.wait_ge(sE, 4 * (c + 1))
                g.dma_start(out=out_d[:, 4096 * c:4096 * (c + 1)],
                            in_=osb[:, 4096 * c:4096 * (c + 1)]).then_inc(sO, 16)
            g.wait_ge(sO, 64)

        @block.tensor
        def _(t):
            t.sem_clear(sL)
            t.sem_clear(sK)
            t.sem_clear(sE)
            t.wait_ge(sL, 48)
            t.wait_ge(sK, 8)
            for k in range(NCH):
                if k >= 2:
                    t.wait_ge(sE, k - 1)
                c0 = 1024 * k
                for jc in range(2):
                    for h in range(2):
                        mm = t.matmul(
                            out=mp[k % 2][:, 512 * h:512 * (h + 1)],
                            lhsT=uT[:, jc, :],
                            rhs=k2[:, jc * FD + c0 + 512 * h:
                                   jc * FD + c0 + 512 * (h + 1)],
                            start=(jc == 0), stop=(jc == 1))
                mm.then_inc(sM, 1)

        @block.vector
        def _(v):
            v.sem_clear(sL)
            v.sem_clear(sA)
            v.sem_clear(sM)
            v.wait_ge(sL, 48)
            for q in range(8):
                v.wait_ge(sA, q + 1)
                jc = q // 4
                v.tensor_mul(
                    k2q[:, q, :].rearrange("k (i d) -> k i d", d=D),
                    grep[:, q % 2, :].rearrange("k (i d) -> k i d", d=D),
                    qkv2[:, jc, :].unsqueeze(1).broadcast_to([128, 64, D]),
                ).then_inc(sK, 1)
            for k in range(NCH):
                v.wait_ge(sM, k + 1)
                c0 = 1024 * k
                v.tensor_mul(
                    osb[:, c0:c0 + 1024].rearrange("b (i d) -> b i d", d=D),
                    mp[k % 2].rearrange("b (i d) -> b i d", d=D),
                    t3[:, 16 * k:16 * (k + 1), :].broadcast_to([128, 16, D]),
                ).then_inc(sE, 1)

        @block.scalar
        def _(a):
            a.sem_clear(sL)
            a.sem_clear(sK)
            a.wait_ge(sL, 16)
            for q in range(8):
                if q >= 2:
                    a.wait_ge(sK, q - 1)
                jc, ih = q // 4, q % 4
                a.copy(out=grep[:, q % 2, :].rearrange("k (i d) -> k i d", d=D),
                       in_=gt[:, jc, 64 * ih:64 * (ih + 1)]
                       .unsqueeze(2).broadcast_to([128, 64, D]))
                a.copy(out=grep[0:1, q % 2, 0:1],
                       in_=grep[0:1, q % 2, 0:1]).then_inc(sA, 1)

    return nc


def _make_in_maps(feature, consts, qkv):
    t, uT = _host_tu(feature, qkv)
    in_maps = [{"consts": consts,
                "tvec": np.ascontiguousarray(t[c * BL:(c + 1) * BL]),
                "uT": uT[c]} for c in range(NCORES)]
    return in_maps, t


def kernel(feature, indicator, W_qk, W_qkv):
    global _compiled
    _setup_jax_cache()
    from concourse.bass_utils import run_bass_kernel_spmd

    consts, qkv = _host_precompute(indicator, W_qk, W_qkv)
    if _compiled is None:
        _compiled = _build_bass()
    nc = _compiled

    in_maps, _t = _make_in_maps(feature, consts, qkv)
    res = run_bass_kernel_spmd(nc, in_maps, list(range(NCORES)))
    out = np.concatenate(
        [r["out"].astype(np.float32).reshape(BL, F, D) for r in res.results],
        axis=0)
    return out



# revision 2
# speedup vs baseline: 19305.6882x; 19305.6882x over previous
"""Trainium2 raw-Bass kernel for nn_InteractionPruningLayer (sparse_attention).

Math (B=1024, F=256, D=64):
    qkv   = einsum('fd,nde->nfe', indicator, W_qkv)            # [3,F,D]
    gate  = (trans[0] @ trans[1].T > 0);  G = (qkv1 @ qkv0.T) * gate
    s[n,b,f] = feature[b,f,:] . qkv[n,f,:];  t = s0*s2;  u = s1
    out[b,i,:] = t[b,i] * sum_j u[b,j] * G[i,j] * qkv2[j,:]

Split of work:
    host   — weight prep (K2[j,i,d] = G[i,j]*qkv2[j,d]) and the per-(b,f)
             projections t/u (~1.7% of FLOPs)
    device — 8 cores sharded along the FEATURE dim i (32 features/core):
             each core contracts the full batch against its own 1MB slice
             of K2 (536M MACs/core, 98% of FLOPs):
                 psum[b,(i,d)] = sum_j uT[j,b] * w[j,(i,d)]    (PE, bf16)
                 out[b,(i,d)]  = t[b,i] * psum[b,(i,d)]        (DVE drain, bf16)
             i-sharding means K2 is *sharded*, not replicated: no on-chip
             G-broadcast build, tiny input DMAs, and the matmul pipeline
             starts as soon as ~1.5MB of inputs land.

Raw bass blocks + explicit semaphores. Semaphore state survives across
executions on these long-lived axon terminals, so each engine zeroes the
sems it waits on first and gpsimd delays its first DMA (~7us of NOPs,
engines complete their clears by ~4us) so no DMA-completion increment
can precede the clears. Output DMAs alternate between the gpsimd and
scalar rings (two hardware queues) to overlap with compute.
"""

import numpy as np
import ml_dtypes

B, F, D = 1024, 256, 64
NCORES = 8
ISH = F // NCORES          # 32 features per core
CW = ISH * D               # 2048 psum/output columns per core
NBC = B // 128             # 8 batch chunks of 128 rows
_compiled = None


def _setup_jax_cache():
    import jax
    try:
        if jax.config.jax_compilation_cache_dir is None:
            jax.config.update("jax_compilation_cache_dir",
                              "/tmp/bass_jax_cache")
            jax.config.update("jax_persistent_cache_min_entry_size_bytes", -1)
            jax.config.update("jax_persistent_cache_min_compile_time_secs", 0)
    except Exception:
        pass


def _host_precompute(indicator, W_qk, W_qkv):
    """Returns K2 [128,2,F,D] f32 packed (jl, jc, i, d) and qkv [3,F,D]."""
    ind = np.asarray(indicator, dtype=np.float32)
    qkv = np.einsum('fd,nde->nfe', ind, np.asarray(W_qkv, dtype=np.float32))
    trans = np.einsum('fd,nde->nfe', ind, np.asarray(W_qk, dtype=np.float32))
    gate = (trans[0] @ trans[1].T) > 0
    G = np.where(gate, qkv[1] @ qkv[0].T, np.float32(0.0)).astype(np.float32)
    # K2[j, i, d] = G[i, j] * qkv2[j, d], packed as [jl, jc, i, d]
    K2 = (G.T[:, :, None] * qkv[2][:, None, :])            # [j, i, d]
    K2 = K2.reshape(2, 128, F, D).transpose(1, 0, 2, 3)    # [jl, jc, i, d]
    return np.ascontiguousarray(K2), qkv


def _host_tu(feature, qkv):
    """t = s0*s2 (f32 [B,F]), uT packed [jl, jc, b] bf16 [128,2,B]."""
    f = np.asarray(feature, dtype=np.float32)
    s = np.einsum('bfd,nfd->nbf', f, qkv, optimize=True)
    t = (s[0] * s[2]).astype(np.float32)                   # [B, F]
    u = s[1].astype(ml_dtypes.bfloat16)                    # [B, F]
    uT = np.ascontiguousarray(
        u.T.reshape(2, 128, B).transpose(1, 0, 2))         # [128, 2, B]
    return t, uT


def _build_bass():
    import concourse.bass as bass
    from concourse import mybir

    nc = bass.Bass()
    f32, bf16 = mybir.dt.float32, mybir.dt.bfloat16

    u_d = nc.declare_dram_parameter("uT", [128, 2 * B], bf16, isOutput=False)
    w_d = nc.declare_dram_parameter("wK2", [128, 2 * CW], bf16, isOutput=False)
    t_d = nc.declare_dram_parameter("tvec", [128, NBC * ISH], f32, isOutput=False)
    out_d = nc.declare_dram_parameter("out", [B, CW], bf16, isOutput=True)

    u_sb = nc.alloc_sbuf_tensor("u_sb", [128, 2 * B], bf16).ap()
    w_sb = nc.alloc_sbuf_tensor("w_sb", [128, 2 * CW], bf16).ap()
    t_sb = nc.alloc_sbuf_tensor("t_sb", [128, NBC * ISH], f32).ap()
    osb = nc.alloc_sbuf_tensor("osb", [128, NBC * CW], bf16).ap()
    mp = [nc.alloc_psum_tensor(f"mp{i}", [128, CW], f32).ap() for i in range(2)]

    u2 = u_sb.rearrange("k (c b) -> k c b", c=2)           # [jl, jc, b]
    w2 = w_sb.rearrange("k (c x) -> k c x", c=2)           # [jl, jc, (i d)]
    t3 = t_sb.rearrange("b (c i) -> b c i", c=NBC)         # [bl, bc, il]

    with (
        nc.Block() as block,
        nc.semaphore("sU") as sU,
        nc.semaphore("sW") as sW,
        nc.semaphore("sT") as sT,
        nc.semaphore("sM") as sM,
        nc.semaphore("sE") as sE,
        nc.semaphore("sO") as sO,
    ):
        @block.gpsimd
        def _(g):
            g.sem_clear(sE)
            g.sem_clear(sO)
            for _ in range(2):
                g.nop(cycle_cnt=4200)
            g.dma_start(out=u_sb[:], in_=u_d[:]).then_inc(sU, 16)
            g.dma_start(out=w_sb[:, 0:CW],
                        in_=w_d[:, 0:CW]).then_inc(sW, 16)
            g.dma_start(out=w_sb[:, CW:2 * CW],
                        in_=w_d[:, CW:2 * CW]).then_inc(sW, 16)
            g.dma_start(out=t_sb[:], in_=t_d[:]).then_inc(sT, 16)
            for bc in range(0, NBC, 2):
                g.wait_ge(sE, bc + 1)
                g.dma_start(out=out_d[128 * bc:128 * (bc + 1), :],
                            in_=osb[:, CW * bc:CW * (bc + 1)]).then_inc(sO, 16)
            g.wait_ge(sO, 16 * NBC)

        @block.scalar
        def _(a):
            a.sem_clear(sE)
            for bc in range(1, NBC, 2):
                a.wait_ge(sE, bc + 1)
                a.dma_start(out=out_d[128 * bc:128 * (bc + 1), :],
                            in_=osb[:, CW * bc:CW * (bc + 1)]).then_inc(sO, 16)

        @block.tensor
        def _(t):
            t.sem_clear(sU)
            t.sem_clear(sW)
            t.sem_clear(sE)
            t.wait_ge(sU, 16)
            for bc in range(NBC):
                if bc >= 2:
                    t.wait_ge(sE, bc - 1)
                for jc in range(2):
                    if bc == 0:
                        t.wait_ge(sW, 16 * (jc + 1))
                    for h in range(4):
                        mm = t.matmul(
                            out=mp[bc % 2][:, 512 * h:512 * (h + 1)],
                            lhsT=u2[:, jc, 128 * bc:128 * (bc + 1)],
                            rhs=w2[:, jc, 512 * h:512 * (h + 1)],
                            start=(jc == 0), stop=(jc == 1))
                mm.then_inc(sM, 1)

        @block.vector
        def _(v):
            v.sem_clear(sT)
            v.sem_clear(sM)
            v.wait_ge(sT, 16)
            for bc in range(NBC):
                v.wait_ge(sM, bc + 1)
                v.tensor_mul(
                    osb[:, CW * bc:CW * (bc + 1)].rearrange(
                        "b (i d) -> b i d", d=D),
                    mp[bc % 2].rearrange("b (i d) -> b i d", d=D),
                    t3[:, bc, :].unsqueeze(2).broadcast_to([128, ISH, D]),
                ).then_inc(sE, 1)

    return nc


def _make_in_maps(feature, K2, qkv):
    t, uT = _host_tu(feature, qkv)
    # t packed per core: t_d[p, bc*ISH + il] = t[bc*128 + p, c*ISH + il]
    tp = t.reshape(NBC, 128, F).transpose(1, 0, 2)         # [p, bc, F]
    u_flat = np.ascontiguousarray(uT.reshape(128, 2 * B))
    in_maps = []
    for c in range(NCORES):
        wc = np.ascontiguousarray(
            K2[:, :, c * ISH:(c + 1) * ISH, :].reshape(128, 2 * CW)
        ).astype(ml_dtypes.bfloat16)
        tc = np.ascontiguousarray(
            tp[:, :, c * ISH:(c + 1) * ISH].reshape(128, NBC * ISH))
        in_maps.append({"uT": u_flat, "wK2": wc, "tvec": tc})
    return in_maps


def kernel(feature, indicator, W_qk, W_qkv):
    global _compiled
    _setup_jax_cache()
    from concourse.bass_utils import run_bass_kernel_spmd

    K2, qkv = _host_precompute(indicator, W_qk, W_qkv)
    if _compiled is None:
        _compiled = _build_bass()
    nc = _compiled

    in_maps = _make_in_maps(feature, K2, qkv)
    res = run_bass_kernel_spmd(nc, in_maps, list(range(NCORES)))
    out = np.empty((B, F, D), dtype=np.float32)
    for c in range(NCORES):
        out[:, c * ISH:(c + 1) * ISH, :] = (
            res.results[c]["out"].astype(np.float32).reshape(B, ISH, D))
    return out


# revision 20
# speedup vs baseline: 24206.1816x; 1.2538x over previous
"""Trainium2 raw-Bass kernel for nn_InteractionPruningLayer (sparse_attention).

Math (B=1024, F=256, D=64):
    qkv   = einsum('fd,nde->nfe', indicator, W_qkv)            # [3,F,D]
    gate  = (trans[0] @ trans[1].T > 0);  G = (qkv1 @ qkv0.T) * gate
    s[n,b,f] = feature[b,f,:] . qkv[n,f,:];  t = s0*s2;  u = s1
    out[b,i,:] = t[b,i] * sum_j u[b,j] * G[i,j] * qkv2[j,:]

Split of work:
    host   — weight prep (K2[j,i,d] = G[i,j]*qkv2[j,d]), the per-(b,f)
             projections t/u, and the t-scale for the 8 features/core the
             scalar engine drains (~2% of FLOPs total)
    device — 8 cores sharded along the FEATURE dim i (32 features/core):
             each core contracts the full batch against its own 1MB slice
             of K2 (536M MACs/core, 98% of FLOPs):
                 psum[b,(i,d)] = sum_j uT[j,b] * w[j,(i,d)]    (PE, bf16)
                 out[b,(i,d)]  = t[b,i] * psum[b,(i,d)]        (drain)
             i-sharding means K2 is *sharded*, not replicated: no on-chip
             G-broadcast build and tiny input DMAs.

Engine orchestration:
  - input DMAs on four rings (gpsimd: u-jc0 + t, vector: u-jc1,
    scalar: w-jc0, sync: w-jc1) so the ~1.7MB fill overlaps; matmuls
    start once u-jc0/w-jc0 land
  - PSUM double-buffered [128,2048]x2; 8 batch-chunks of 128 rows
  - PSUM drain split: vector does cols 0:1536 with the fused t-multiply
    (tensor_tensor); scalar does cols 1536:2048 as a plain copy-cast
    (ACT cannot tensor-multiply; host applies t for those 8 features)
    so neither engine exceeds the 8-matmul PE cadence per chunk
  - output DMAs alternate between the scalar and sync rings

Cross-execution semaphore safety: sem state survives across NEFF
executions on these long-lived axon terminals. The kernel postamble
clears the whole sem range, so only the FIRST execution after a foreign
NEFF sees garbage. Defenses: (1) gpsimd dma_reset + sem_clear over the
whole kernel range first, (2) every engine re-clears each semaphore it
waits on before its first wait (clear->wait on the same engine is
race-free), and the framework's init all-engine barrier plus >=4us DMA
latency guarantees no completion increment can precede those clears.
"""

import numpy as np
import ml_dtypes

B, F, D = 1024, 256, 64
NCORES = 8
ISH = F // NCORES          # 32 features per core
CW = ISH * D               # 2048 psum/output columns per core
NBC = B // 128             # 8 batch chunks of 128 rows
VC = 1536                  # vector-drain cols (PSUM-bank aligned: 3x512)
HI = VC // D               # first host-side feature index of the raw slice
_compiled = None


def _setup_jax_cache():
    import jax
    try:
        if jax.config.jax_compilation_cache_dir is None:
            jax.config.update("jax_compilation_cache_dir",
                              "/tmp/bass_jax_cache")
            jax.config.update("jax_persistent_cache_min_entry_size_bytes", -1)
            jax.config.update("jax_persistent_cache_min_compile_time_secs", 0)
    except Exception:
        pass


def _host_precompute(indicator, W_qk, W_qkv):
    """Returns K2 [128,2,F,D] f32 packed (jl, jc, i, d) and qkv [3,F,D]."""
    ind = np.asarray(indicator, dtype=np.float32)
    qkv = np.einsum('fd,nde->nfe', ind, np.asarray(W_qkv, dtype=np.float32))
    trans = np.einsum('fd,nde->nfe', ind, np.asarray(W_qk, dtype=np.float32))
    gate = (trans[0] @ trans[1].T) > 0
    G = np.where(gate, qkv[1] @ qkv[0].T, np.float32(0.0)).astype(np.float32)
    # K2[j, i, d] = G[i, j] * qkv2[j, d], packed as [jl, jc, i, d]
    K2 = (G.T[:, :, None] * qkv[2][:, None, :])            # [j, i, d]
    K2 = K2.reshape(2, 128, F, D).transpose(1, 0, 2, 3)    # [jl, jc, i, d]
    return np.ascontiguousarray(K2), qkv


def _host_tu(feature, qkv):
    """t = s0*s2 (f32 [B,F]), uT packed [jl, bc, jc, bl] bf16 [128,8,2,128]."""
    f = np.asarray(feature, dtype=np.float32)
    s = np.einsum('bfd,nfd->nbf', f, qkv, optimize=True)
    t = (s[0] * s[2]).astype(np.float32)                   # [B, F]
    u = s[1].astype(ml_dtypes.bfloat16)                    # [B, F]
    uT = np.ascontiguousarray(                             # [jl, bc, jc, bl]
        u.T.reshape(2, 128, NBC, 128).transpose(1, 2, 0, 3))
    return t, uT


def _build_bass():
    import concourse.bass as bass
    from concourse import mybir

    nc = bass.Bass()
    f32, bf16 = mybir.dt.float32, mybir.dt.bfloat16

    u_d = nc.declare_dram_parameter("uT", [128, 2 * B], bf16, isOutput=False)
    w_d = nc.declare_dram_parameter("wK2", [128, 2 * CW], bf16, isOutput=False)
    t_d = nc.declare_dram_parameter("tvec", [128, NBC * ISH], f32, isOutput=False)
    out_d = nc.declare_dram_parameter("out", [B, CW], bf16, isOutput=True)

    u_sb = nc.alloc_sbuf_tensor("u_sb", [128, 2 * B], bf16).ap()
    w_sb = nc.alloc_sbuf_tensor("w_sb", [128, 2 * CW], bf16).ap()
    t_sb = nc.alloc_sbuf_tensor("t_sb", [128, NBC * ISH], f32).ap()
    osb = nc.alloc_sbuf_tensor("osb", [128, NBC * CW], bf16).ap()
    mp = [nc.alloc_psum_tensor(f"mp{i}", [128, CW], f32).ap() for i in range(2)]

    u2 = u_sb.rearrange("k (c j b) -> k c j b", c=NBC, j=2)  # [jl, bc, jc, bl]
    w2 = w_sb.rearrange("k (c x) -> k c x", c=2)           # [jl, jc, (i d)]
    t3 = t_sb.rearrange("b (c i) -> b c i", c=NBC)         # [bl, bc, il]

    # Kernel-start semaphore reset (the target_bir_lowering=False path does
    # not emit one): gpsimd resets DMA state + zeroes every non-barrier
    # kernel semaphore. Runs before any engine's user body issues DMAs.
    for rng in bass.compact_to_ranges(
            [s for s in nc._kernel_sem_range if s not in nc.barrier_sems]):
        nc.gpsimd.dma_reset(rng)
        nc.gpsimd.sem_clear(rng)

    NUC = 4                    # u DMA chunks (2 batch-chunks each)
    UCW = 2 * B // NUC         # 512 u columns per chunk

    with (
        nc.Block() as block,
        nc.semaphore("sU0") as sU0,
        nc.semaphore("sU1") as sU1,
        nc.semaphore("sU2") as sU2,
        nc.semaphore("sU3") as sU3,
        nc.semaphore("sWa") as sWa,
        nc.semaphore("sWb") as sWb,
        nc.semaphore("sT") as sT,
        nc.semaphore("sM") as sM,
        nc.semaphore("sE") as sE,
        nc.semaphore("sC") as sC,
        nc.semaphore("sO") as sO,
    ):
        sU = [sU0, sU1, sU2, sU3]

        @block.gpsimd
        def _(g):
            g.sem_clear(sO)
            for k in range(NUC):
                g.dma_start(out=u_sb[:, UCW * k:UCW * (k + 1)],
                            in_=u_d[:, UCW * k:UCW * (k + 1)]
                            ).then_inc(sU[k], 16)
            g.dma_start(out=t_sb[:], in_=t_d[:]).then_inc(sT, 16)
            g.wait_ge(sO, 16 * NBC)

        @block.vector
        def _(v):
            v.sem_clear(sT)
            v.sem_clear(sM)
            v.wait_ge(sT, 16)
            for bc in range(NBC):
                v.wait_ge(sM, bc + 1)
                v.tensor_mul(
                    osb[:, CW * bc:CW * bc + VC].rearrange(
                        "b (i d) -> b i d", d=D),
                    mp[bc % 2][:, 0:VC].rearrange("b (i d) -> b i d", d=D),
                    t3[:, bc, 0:HI].unsqueeze(2).broadcast_to([128, HI, D]),
                ).then_inc(sE, 1)

        @block.scalar
        def _(a):
            a.sem_clear(sM)
            a.sem_clear(sE)
            a.dma_start(out=w_sb[:, 0:CW], in_=w_d[:, 0:CW]).then_inc(sWa, 16)
            for bc in range(NBC):
                a.wait_ge(sM, bc + 1)
                a.copy(out=osb[:, CW * bc + VC:CW * (bc + 1)],
                       in_=mp[bc % 2][:, VC:CW]).then_inc(sC, 1)
                if bc % 2 == 0:
                    a.wait_ge(sE, bc + 1)
                    a.dma_start(out=out_d[128 * bc:128 * (bc + 1), :],
                                in_=osb[:, CW * bc:CW * (bc + 1)]
                                ).then_inc(sO, 16)

        @block.sync
        def _(sp):
            sp.sem_clear(sE)
            sp.sem_clear(sC)
            sp.dma_start(out=w_sb[:, CW:2 * CW],
                         in_=w_d[:, CW:2 * CW]).then_inc(sWb, 16)
            for bc in range(1, NBC, 2):
                sp.wait_ge(sE, bc + 1)
                sp.wait_ge(sC, bc + 1)
                sp.dma_start(out=out_d[128 * bc:128 * (bc + 1), :],
                             in_=osb[:, CW * bc:CW * (bc + 1)]).then_inc(sO, 16)

        @block.tensor
        def _(t):
            for s in (sU0, sU1, sU2, sU3, sWa, sWb, sE, sC):
                t.sem_clear(s)
            t.wait_ge(sU0, 16)
            t.wait_ge(sWa, 16)
            for bc in range(NBC):
                if bc % 2 == 0 and bc > 0:
                    t.wait_ge(sU[bc // 2], 16)
                if bc >= 2:
                    t.wait_ge(sE, bc - 1)
                    t.wait_ge(sC, bc - 1)
                for jc in range(2):
                    if bc == 0 and jc == 1:
                        t.wait_ge(sWb, 16)
                    for h in range(4):
                        mm = t.matmul(
                            out=mp[bc % 2][:, 512 * h:512 * (h + 1)],
                            lhsT=u2[:, bc, jc, :],
                            rhs=w2[:, jc, 512 * h:512 * (h + 1)],
                            start=(jc == 0), stop=(jc == 1))
                mm.then_inc(sM, 1)

    return nc


def _make_in_maps(feature, K2, qkv):
    t, uT = _host_tu(feature, qkv)
    # t packed per core: t_d[p, bc*ISH + il] = t[bc*128 + p, c*ISH + il]
    tp = t.reshape(NBC, 128, F).transpose(1, 0, 2)         # [p, bc, F]
    u_flat = np.ascontiguousarray(uT.reshape(128, 2 * B))
    in_maps = []
    for c in range(NCORES):
        wc = np.ascontiguousarray(
            K2[:, :, c * ISH:(c + 1) * ISH, :].reshape(128, 2 * CW)
        ).astype(ml_dtypes.bfloat16)
        tc = np.ascontiguousarray(
            tp[:, :, c * ISH:(c + 1) * ISH].reshape(128, NBC * ISH))
        in_maps.append({"uT": u_flat, "wK2": wc, "tvec": tc})
    return in_maps, t


def kernel(feature, indicator, W_qk, W_qkv):
    global _compiled
    _setup_jax_cache()
    from concourse.bass_utils import run_bass_kernel_spmd

    K2, qkv = _host_precompute(indicator, W_qk, W_qkv)
    if _compiled is None:
        _compiled = _build_bass()
    nc = _compiled

    in_maps, t = _make_in_maps(feature, K2, qkv)
    res = run_bass_kernel_spmd(nc, in_maps, list(range(NCORES)))
    out = np.empty((B, F, D), dtype=np.float32)
    for c in range(NCORES):
        oc = res.results[c]["out"].astype(np.float32).reshape(B, ISH, D)
        # cols HI..ISH came from the scalar copy-cast drain: apply t here
        oc[:, HI:, :] *= t[:, c * ISH + HI:(c + 1) * ISH, None]
        out[:, c * ISH:(c + 1) * ISH, :] = oc
    return out
